# revision 18
# baseline (speedup 1.0000x reference)
"""Chamfer-KL loss kernel for Trainium2 (8 NeuronCores, batch-parallel).

Per core: one batch sample.
  M[i,j] = mu_p[i]@mu_g[j] - 0.5||mu_p[i]||^2 - 0.5||mu_g[j]||^2  (= -dist/2)
computed as a bf16 matmul with the norm terms folded in as extra contraction
rows, the x-side scaled by 2^17 and a rank-1 (ones x magic) row of 1.5*2^35
added (K=35).  PSUM's own fp32 rounding then quantizes every entry to
  psum = MAGIC3 + 4096*rint(32*M)
i.e. an exact multiple of 4096 (plus MAGIC3), leaving the low 12 bits free
to carry a column index exactly.

The per-row argmax is an ORDER-FREE packed value+index max: a custom DVE op
computes  pack = (max(a,b) - MAGIC3) + (2k + off + [b>a])  and MAX-accums it.
Because packs from different chunks compare correctly, the 4096-wide row is
processed as two independent 1024-pair fold ops over 2-bank PSUM chunks,
which double-buffers PSUM (bufs=2 everywhere) and removes the scan->matmul
serialization of a monolithic 8-bank layout.  Per tile: PE fills 4 psum
chunks (row-group pairs), ACT stages 2 chunks to SBUF fp32, DVE folds the
other 2 chunks against the staged ones.  GPSIMD decodes packed indices,
issues the KL gathers, and computes the (exact fp32) KL chunks inline so
almost nothing is left for a tail.
"""

import numpy as np

BS, N, D = 8, 4096, 32
NT = N // 128          # 32 partition tiles
NC = 1024              # fold-chunk width (pairs per custom-op call)
KAUG = D + 3           # 32 features + norm row + ones row + magic row
ALPHA = 2.0**17
GAMMA = 1.5 * 2.0**18  # ALPHA * GAMMA = MAGIC3
MAGIC3 = 1.5 * 2.0**35

_NC_CACHE = {}
_OP_CACHE = {}


def _get_packmax_op():
    """Order-free packed value+index fold-max custom DVE op.

    pack = (max(Src0, Src1) - C0) + (idx2 + [Src1 > Src0])
    with idx2 = C2 + 2k (scan step C1=2, init C2-C1); accum = MAX.
    Inputs carry MAGIC3 + 4096*q so packs are exact fp32 integers."""
    if "op" in _OP_CACHE:
        return _OP_CACHE["op"]
    from concourse.dve_spec import (
        Spec, Src0, Src1, C0, C1, C2, Zero, AluOp, maxx, scan, lower, Bin,
    )
    import concourse.dve_ops as dve_ops
    from concourse.dve_uop import DveOpSpec

    idx2 = scan(AluOp.ADD, C1, init=Bin(AluOp.SUBTRACT, C2, C1))
    c2 = Src1 > Src0
    body = (maxx(Src0, Src1) - C0) + (idx2 + c2)

    def ref(in0, in1, c0, c1, imm2):
        a = np.asarray(in0, np.float32)
        b = np.asarray(in1, np.float32)
        m = np.maximum(a, b)
        k = np.arange(a.shape[-1], dtype=np.float32)
        idx2v = (np.float32(imm2) + np.float32(c1) * k).astype(np.float32)
        bwin = (b > a).astype(np.float32)
        body = ((m - np.float32(c0)) + (idx2v + bwin)).astype(np.float32)
        acc = body.max(axis=-1, keepdims=True)
        return body, acc

    spec = Spec(body=body, accum=AluOp.MAX, reference=ref)
    shas = {
        ver: DveOpSpec(
            name="PACKMAX_FOLD2_ANT", opcode=1, uops=lower(spec, ver=ver),
            rd1_en=True,
        ).sha(ver)
        for ver in ("v3", "v4")
    }
    op = dve_ops.DveOp("PACKMAX_FOLD2_ANT", spec, subdim=False, uops_sha=shas)
    if all(o.name != op.name for o in dve_ops.OPS):
        dve_ops.OPS.append(op)
        dve_ops.CUSTOM_DVE_SPECS[op.name] = op.spec
        dve_ops._SUB_OPCODE_FOR_NAME[op.name] = (
            dve_ops._CUSTOM_DVE_ROW_BASE + len(dve_ops.OPS) - 1
        )
    _OP_CACHE["op"] = op
    return op


def _build():
    from contextlib import ExitStack

    import concourse.mybir as mybir
    from concourse import bacc
    from concourse.bass import IndirectOffsetOnAxis
    from concourse.tile import TileContext

    f32 = mybir.dt.float32
    bf16 = mybir.dt.bfloat16
    u32 = mybir.dt.uint32
    AF = mybir.ActivationFunctionType
    packmax_op = _get_packmax_op()

    nc = bacc.Bacc(None, target_bir_lowering=False)
    xT = nc.dram_tensor("xT", [KAUG, N], bf16, kind="ExternalInput")
    yT = nc.dram_tensor("yT", [KAUG, N], bf16, kind="ExternalInput")
    # gather tables, permuted so the packed index decodes directly to a row
    cat_p2 = nc.dram_tensor("cat_p2", [N, 2 * D], f32, kind="ExternalInput")
    cat_g2 = nc.dram_tensor("cat_g2", [N, 2 * D], f32, kind="ExternalInput")
    # host-pretransposed [p][t][c] natural-order copies
    natp_h = nc.dram_tensor("natp_h", [128, NT * 2 * D], f32, kind="ExternalInput")
    natg_h = nc.dram_tensor("natg_h", [128, NT * 2 * D], f32, kind="ExternalInput")
    loss = nc.dram_tensor("loss", [1, 1], f32, kind="ExternalOutput")

    with TileContext(nc) as tc:
        with ExitStack() as ctx:
            const = ctx.enter_context(tc.tile_pool(name="const", bufs=1))
            staged_pool = ctx.enter_context(tc.tile_pool(name="staged", bufs=6))
            scr_pool = ctx.enter_context(tc.tile_pool(name="scr", bufs=2))
            kl_pool = ctx.enter_context(tc.tile_pool(name="klp", bufs=6))
            small = ctx.enter_context(tc.tile_pool(name="small", bufs=4))
            # 2-bank psum chunks, double buffered: 4+4 banks
            fold_ps = ctx.enter_context(tc.tile_pool(name="fps", bufs=2, space="PSUM"))
            stage_ps = ctx.enter_context(tc.tile_pool(name="sps", bufs=2, space="PSUM"))

            # stationary operands duplicated at base_partition 64: row-group
            # tiling runs two K=35 matmuls concurrently in PE quadrants
            xT_sb = const.tile([64 + KAUG, N], bf16, tag="xT_sb")
            yT_sb = const.tile([64 + KAUG, N], bf16, tag="yT_sb")
            nat_p = const.tile([128, NT, 2 * D], f32, tag="nat_p")
            nat_g = const.tile([128, NT, 2 * D], f32, tag="nat_g")
            gath_g = const.tile([128, NT, 2 * D], f32, tag="gath_g")
            gath_p = const.tile([128, NT, 2 * D], f32, tag="gath_p")
            pargs_a = const.tile([128, 2, NT], f32, tag="pargs_a")
            pargs_b = const.tile([128, 2, NT], f32, tag="pargs_b")
            args_a = const.tile([128, NT], u32, tag="args_a")
            args_b = const.tile([128, NT], u32, tag="args_b")
            klacc = const.tile([128, NT], f32, tag="klacc")

            # --- input loads: column-chunked across 4 DGE queues so the
            # first tile's matmuls start ~2.5us in (vs ~30us serial) ---
            def ld(q, sb, dram, lo, hi, part):
                p0 = 0 if part == 0 else 64
                q.dma_start(out=sb[p0 : p0 + KAUG, lo:hi], in_=dram[:, lo:hi])

            ld(nc.sync,   xT_sb, xT, 0, 512, 0)
            ld(nc.scalar, xT_sb, xT, 0, 512, 1)
            ld(nc.gpsimd, yT_sb, yT, 2048, 3072, 0)
            ld(nc.sync,   yT_sb, yT, 2048, 3072, 1)
            ld(nc.scalar, yT_sb, yT, 3072, 4096, 0)
            ld(nc.gpsimd, yT_sb, yT, 3072, 4096, 1)
            ld(nc.sync,   yT_sb, yT, 0, 1024, 0)
            ld(nc.scalar, yT_sb, yT, 0, 1024, 1)
            ld(nc.gpsimd, yT_sb, yT, 1024, 2048, 0)
            ld(nc.sync,   yT_sb, yT, 1024, 2048, 1)
            ld(nc.scalar, xT_sb, xT, 512, 2048, 0)
            ld(nc.gpsimd, xT_sb, xT, 512, 2048, 1)
            ld(nc.sync,   xT_sb, xT, 2048, 4096, 0)
            ld(nc.scalar, xT_sb, xT, 2048, 4096, 1)
            # nat tables last: they are not needed until the first KL slot
            # (~40us in) and would otherwise steal DMA bandwidth from the
            # pipeline-gating xT/yT loads
            nc.gpsimd.dma_start(out=nat_p[:, :, :], in_=natp_h[:, :])
            nc.sync.dma_start(out=nat_g[:, :, :], in_=natg_h[:, :])

            def mm_pair(ps, stat_sb, mov_sb, tcol, mlo, first):
                """One 1024-col chunk: two concurrent 512-col matmuls in PE
                row groups {0,1} (partitions 0..34) and {2,3} (64..98)."""
                i1 = nc.tensor.matmul(
                    ps[:, 0:512],
                    lhsT=stat_sb[0:KAUG, tcol : tcol + 128],
                    rhs=mov_sb[0:KAUG, mlo : mlo + 512],
                    start=True, stop=True,
                )
                i2 = nc.tensor.matmul(
                    ps[:, 512:1024],
                    lhsT=stat_sb[64 : 64 + KAUG, tcol : tcol + 128],
                    rhs=mov_sb[64 : 64 + KAUG, mlo + 512 : mlo + 1024],
                    start=True, stop=True,
                )
                if not first:
                    i1.ins.ldweights = False
                    i2.ins.ldweights = False

            GT = 8  # tiles per decode/gather/KL group (amortizes Pool's
                    # ~300ns fixed cost per op and the ~1.1us fixed cost
                    # per indirect-DMA instruction)

            MI = 1.5 * 2.0**23       # integer-grid rint magic
            CH = 0.5 - 2.0**-13      # floor shift, tie-safe for |q| < 2048

            def decode(pargs, args, lo, hi, on_vec=False):
                """pargs[:, 0:2, lo:hi] (packed) -> args[:, lo:hi] (u32).
                idx = pack - 4096*floor(pack/4096); floor via the fp32
                magic-number rint of (pack/4096 - CH).  Runs entirely on
                Pool (TT max via relu; the cast via a convert-copy) so no
                in-order Vector-queue slot ever waits on Pool."""
                n = hi - lo
                e = nc.vector if on_vec else nc.gpsimd
                t1 = small.tile([128, n], f32, tag="dt1")
                t2 = small.tile([128, n], f32, tag="dt2")
                if on_vec:
                    e.tensor_max(t1[:, :], pargs[:, 0, lo:hi], pargs[:, 1, lo:hi])
                else:
                    # max(a,b) = a + relu(b - a); Pool has no TT-max opcode
                    e.tensor_sub(t1[:, :], pargs[:, 1, lo:hi], pargs[:, 0, lo:hi])
                    e.tensor_relu(t1[:, :], t1[:, :])
                    e.tensor_add(t1[:, :], t1[:, :], pargs[:, 0, lo:hi])
                e.tensor_scalar(
                    t2[:, :], t1[:, :], 1.0 / 4096.0, CH,
                    op0=mybir.AluOpType.mult, op1=mybir.AluOpType.subtract,
                )
                e.tensor_scalar(
                    t2[:, :], t2[:, :], MI, MI,
                    op0=mybir.AluOpType.add, op1=mybir.AluOpType.subtract,
                )
                e.tensor_scalar_mul(t2[:, :], t2[:, :], -4096.0)
                e.tensor_add(t2[:, :], t1[:, :], t2[:, :])
                e.tensor_copy(args[:, lo:hi], t2[:, :])

            def gathers(args, table, gath, lo, hi):
                # per-tile gathers: multi-index offset APs gather garbage on
                # real DGE hardware (verified on-device)
                for j in range(lo, hi):
                    nc.gpsimd.indirect_dma_start(
                        gath[:, j, :],
                        None,
                        table[:, :],
                        IndirectOffsetOnAxis(ap=args[:, j : j + 1], axis=0),
                    )

            # KL chain for one group, staged so every op's producers ran
            # periods earlier (no head-of-line blocking on in-order queues):
            #   S = sum_d (t1 - exp(t1) - (mu_p-mu_o)^2 * exp(-lv_o))
            kl_state = {}

            def kl1(pv, ov, key, lo, hi, vec):
                e = nc.vector if vec else nc.gpsimd
                n = hi - lo
                k1 = kl_pool.tile([128, n, D], f32, tag="k1")
                kl_state[key] = (k1,)
                e.tensor_sub(k1[:, :, :], pv[:, lo:hi, D : 2 * D], ov[:, lo:hi, D : 2 * D])

            def kl2(pv, ov, key, lo, hi, vec):
                (k1,) = kl_state[key]
                n = hi - lo
                k2 = kl_pool.tile([128, n, D], f32, tag="k2")
                k3 = kl_pool.tile([128, n, D], f32, tag="k3")
                nc.scalar.activation(k2[:, :, :], k1[:, :, :], AF.Exp)
                nc.scalar.activation(k3[:, :, :], ov[:, lo:hi, D : 2 * D], AF.Exp, scale=-1.0)
                kl_state[key] = (k1, k2, k3)

            def kl3(pv, ov, key, lo, hi, vec):
                k1, k2, k3 = kl_state[key]
                e = nc.vector if vec else nc.gpsimd
                n = hi - lo
                k4 = kl_pool.tile([128, n, D], f32, tag="k4")
                e.tensor_sub(k1[:, :, :], k1[:, :, :], k2[:, :, :])
                e.tensor_sub(k2[:, :, :], pv[:, lo:hi, 0:D], ov[:, lo:hi, 0:D])
                e.tensor_mul(k2[:, :, :], k2[:, :, :], k2[:, :, :])
                e.tensor_mul(k2[:, :, :], k2[:, :, :], k3[:, :, :])
                e.tensor_sub(k4[:, :, :], k1[:, :, :], k2[:, :, :])
                kl_state[key] = (k4,)

            def kl4(key, lo, hi, first):
                (k4,) = kl_state.pop(key)
                n = hi - lo
                if first:
                    nc.vector.reduce_sum(
                        klacc[:, lo:hi], k4[:, :, :], axis=mybir.AxisListType.X
                    )
                else:
                    red = small.tile([128, n], f32, tag="red")
                    nc.vector.reduce_sum(
                        red[:, :], k4[:, :, :], axis=mybir.AxisListType.X
                    )
                    nc.gpsimd.tensor_add(klacc[:, lo:hi], klacc[:, lo:hi], red[:, :])

            # --- two layout passes, one global slot-scheduled pipeline ---
            LAYOUTS = (
                (xT_sb, yT_sb, pargs_a, args_a, cat_g2, gath_g, nat_p, gath_g, True),
                (yT_sb, xT_sb, pargs_b, args_b, cat_p2, gath_p, gath_p, nat_g, False),
            )
            sched = {}

            def at(T, fn):
                sched.setdefault(T, []).append(fn)

            NTOT = 2 * NT
            for L, (stat_sb, mov_sb, pargs, args, table, gath, pv, ov, first_kl) in (
                enumerate(LAYOUTS)
            ):
                for t in range(NT):
                    T = L * NT + t
                    tcol = t * 128
                    # stage-half chunks (cols 2048:4096) -> psum -> ACT -> SBUF
                    st = []
                    for c in range(2):
                        sps = stage_ps.tile([128, NC], f32, tag="sps")
                        mm_pair(sps, stat_sb, mov_sb, tcol, 2048 + NC * c, first=(c == 0))
                        st.append(sps)
                    sg = []
                    for c in range(2):
                        sb = staged_pool.tile([128, NC], f32, tag="staged")
                        nc.scalar.copy(out=sb[:, :], in_=st[c][:, :])
                        sg.append(sb)
                    # fold-half chunks (cols 0:2048) stay in PSUM
                    for c in range(2):
                        fps = fold_ps.tile([128, NC], f32, tag="fps")
                        mm_pair(fps, stat_sb, mov_sb, tcol, NC * c, first=False)
                        scr = scr_pool.tile([128, NC], f32, tag="scr")
                        nc.vector._custom_dve(
                            packmax_op,
                            out=scr[:, :],
                            in0=fps[:, :],
                            in1=sg[c][:, :],
                            s0=MAGIC3,
                            s1=2.0,
                            imm2=float(2048 * c),
                            accum_out=pargs[:, c, t : t + 1],
                        )
                    # run work scheduled for this slot (producers long done)
                    for fn in sched.pop(T, []):
                        fn()
                    in_last_group = L == 1 and t >= NT - GT
                    if t % GT == GT - 1 and not in_last_group:
                        g = t // GT
                        lo, hi = GT * g, GT * g + GT
                        decode(pargs, args, lo, hi)
                        gathers(args, table, gath, lo, hi)
                        key = (L, g)
                        P, O, F = pv, ov, first_kl
                        at(T + 4, lambda P=P, O=O, k=key, a=lo, b=hi: kl1(P, O, k, a, b, False))
                        at(T + 6, lambda P=P, O=O, k=key, a=lo, b=hi: kl2(P, O, k, a, b, False))
                        at(T + 10, lambda P=P, O=O, k=key, a=lo, b=hi: kl3(P, O, k, a, b, False))
                        at(T + 16, lambda k=key, a=lo, b=hi, F=F: kl4(k, a, b, F))
                    elif in_last_group and t % 4 == 3:
                        # final group: split gathers in half and run the KL
                        # on the (soon idle) Vector engine to cut the tail
                        half = (t % GT) // 4
                        lo = NT - GT + 4 * half
                        hi = lo + 4
                        decode(pargs, args, lo, hi, on_vec=(half == 1))
                        gathers(args, table, gath, lo, hi)

            # drain: remaining scheduled KL work, then the last group's KL
            # entirely on Vector/ACT (Pool is still busy with its gathers)
            for T in range(NTOT, NTOT + 17):
                for fn in sched.pop(T, []):
                    fn()
            stat_sb, mov_sb, pargs, args, table, gath, pv, ov, first_kl = LAYOUTS[1]
            lo, hi = GT * 3, GT * 4
            key = (1, 3)
            kl1(pv, ov, key, lo, hi, True)
            kl2(pv, ov, key, lo, hi, True)
            kl3(pv, ov, key, lo, hi, True)
            kl4(key, lo, hi, first_kl)

            # --- final reduction ---
            # fold the two "+ sum_d 1 = +D" constants (one per side)
            nc.vector.tensor_scalar_add(klacc[:, :], klacc[:, :], float(2 * D))
            ones_col = const.tile([128, 1], f32, tag="ones_col")
            nc.vector.memset(ones_col[:, :], 1.0)
            ps_fin = stage_ps.tile([128, NC], f32, tag="sps")
            nc.tensor.matmul(
                ps_fin[0:1, 0:NT],
                lhsT=ones_col[:, :],
                rhs=klacc[:, :],
                start=True, stop=True,
            )
            fin = small.tile([1, 1], f32, tag="fin")
            nc.vector.reduce_sum(
                fin[:, :], ps_fin[0:1, 0:NT], axis=mybir.AxisListType.X
            )
            # loss = 0.5*(l1+l2), each l = -0.5*S  ->  -0.25*(S1+S2)
            nc.vector.tensor_scalar_mul(fin[:, :], fin[:, :], -0.25)
            nc.sync.dma_start(out=loss[:, :], in_=fin[:, :])

    nc.finalize()
    return nc


def _get_nc():
    if "nc" not in _NC_CACHE:
        _NC_CACHE["nc"] = _build()
    return _NC_CACHE["nc"]


def _host_prep(mu_p, lv_p, mu_g, lv_g):
    """Per-sample input marshalling.

    xT carries the 2^17 scale (exact exponent shift in bf16) and yT the
    magic partner row, so psum = MAGIC3 + 2^17*M rounded to a 4096 grid.
    Gather tables are permuted so packed idx2 = 2048c + 2q + b maps to
    row idx2 directly (orig col = 1024c + q + 2048b)."""
    import ml_dtypes

    bf16 = ml_dtypes.bfloat16
    x = mu_p.astype(bf16)
    y = mu_g.astype(bf16)
    xf = x.astype(np.float32)
    yf = y.astype(np.float32)
    ax = (-0.5 * np.sum(xf * xf, -1)).astype(bf16).astype(np.float32)
    ay = (-0.5 * np.sum(yf * yf, -1)).astype(bf16).astype(np.float32)
    ones = np.ones((N,), np.float32)
    xT = np.ascontiguousarray(
        (np.concatenate(
            [xf.T, ax[None, :], ones[None, :], ones[None, :]], 0
        ) * np.float32(ALPHA)).astype(bf16)
    )
    yT = np.ascontiguousarray(
        np.concatenate(
            [yf.T, ones[None, :], ay[None, :], np.full((1, N), GAMMA, np.float32)], 0
        ).astype(bf16)
    )
    P = np.arange(N)
    c = P >> 11
    r = P & 2047
    orig = 1024 * c + (r >> 1) + 2048 * (r & 1)
    cat_p = np.concatenate([mu_p, lv_p], 1).astype(np.float32)
    cat_g = np.concatenate([mu_g, lv_g], 1).astype(np.float32)
    natp_h = np.ascontiguousarray(
        cat_p.reshape(NT, 128, 2 * D).transpose(1, 0, 2).reshape(128, -1)
    )
    natg_h = np.ascontiguousarray(
        cat_g.reshape(NT, 128, 2 * D).transpose(1, 0, 2).reshape(128, -1)
    )
    return {
        "xT": xT,
        "yT": yT,
        "cat_p2": np.ascontiguousarray(cat_p[orig]),
        "cat_g2": np.ascontiguousarray(cat_g[orig]),
        "natp_h": natp_h,
        "natg_h": natg_h,
    }


def make_in_maps(mu_preds, logvar_preds, mu_gts, logvar_gts):
    mu_preds = np.asarray(mu_preds, dtype=np.float32)
    logvar_preds = np.asarray(logvar_preds, dtype=np.float32)
    mu_gts = np.asarray(mu_gts, dtype=np.float32)
    logvar_gts = np.asarray(logvar_gts, dtype=np.float32)
    return [
        _host_prep(mu_preds[b], logvar_preds[b], mu_gts[b], logvar_gts[b])
        for b in range(BS)
    ]


def run(in_maps, trace=False):
    from concourse.bass_utils import run_bass_kernel_spmd

    nc = _get_nc()
    res = run_bass_kernel_spmd(nc, in_maps, list(range(BS)), trace=trace)
    out = np.array(
        [np.asarray(res.results[b]["loss"]).reshape(()) for b in range(BS)],
        dtype=np.float32,
    )
    return out, res


def kernel(mu_preds, logvar_preds, mu_gts, logvar_gts):
    in_maps = make_in_maps(mu_preds, logvar_preds, mu_gts, logvar_gts)
    out, _ = run(in_maps)
    return out


# revision 20
# speedup vs baseline: 1.1807x; 1.1807x over previous
"""Chamfer-KL loss kernel for Trainium2 (8 NeuronCores, batch-parallel).

Per core: one batch sample.
  M[i,j] = mu_p[i]@mu_g[j] - 0.5||mu_p[i]||^2 - 0.5||mu_g[j]||^2  (= -dist/2)
computed as a bf16 matmul with the norm terms folded in as extra contraction
rows, the x-side scaled by 2^17 and a rank-1 (ones x magic) row of 1.5*2^35
added (K=35).  PSUM's own fp32 rounding then quantizes every entry to
  psum = MAGIC3 + 4096*rint(32*M)
i.e. an exact multiple of 4096 (plus MAGIC3), leaving the low 12 bits free
to carry a column index exactly.

The per-row argmax is an ORDER-FREE packed value+index max: a custom DVE op
computes  pack = (max(a,b) - MAGIC3) + (2k + off + [b>a])  and MAX-accums it.
Because packs from different chunks compare correctly, the 4096-wide row is
processed as two independent 1024-pair fold ops over 2-bank PSUM chunks,
which double-buffers PSUM (bufs=2 everywhere) and removes the scan->matmul
serialization of a monolithic 8-bank layout.  Per tile: PE fills 4 psum
chunks (row-group pairs), ACT stages 2 chunks to SBUF fp32, DVE folds the
other 2 chunks against the staged ones.  GPSIMD decodes packed indices,
issues the KL gathers, and computes the (exact fp32) KL chunks inline so
almost nothing is left for a tail.
"""

import numpy as np

BS, N, D = 8, 4096, 32
NT = N // 128          # 32 partition tiles
NC = 1024              # fold-chunk width (pairs per custom-op call)
KAUG = D + 3           # 32 features + norm row + ones row + magic row
ALPHA = 2.0**17
GAMMA = 1.5 * 2.0**18  # ALPHA * GAMMA = MAGIC3
MAGIC3 = 1.5 * 2.0**35

_NC_CACHE = {}
_OP_CACHE = {}


def _get_packmax_op():
    """Order-free packed value+index fold-max custom DVE op.

    pack = (max(Src0, Src1) - C0) + (idx2 + [Src1 > Src0])
    with idx2 = C2 + 2k (scan step C1=2, init C2-C1); accum = MAX.
    Inputs carry MAGIC3 + 4096*q so packs are exact fp32 integers."""
    if "op" in _OP_CACHE:
        return _OP_CACHE["op"]
    from concourse.dve_spec import (
        Spec, Src0, Src1, C0, C1, C2, Zero, AluOp, maxx, scan, lower, Bin,
    )
    import concourse.dve_ops as dve_ops
    from concourse.dve_uop import DveOpSpec

    idx2 = scan(AluOp.ADD, C1, init=Bin(AluOp.SUBTRACT, C2, C1))
    c2 = Src1 > Src0
    body = (maxx(Src0, Src1) - C0) + (idx2 + c2)

    def ref(in0, in1, c0, c1, imm2):
        a = np.asarray(in0, np.float32)
        b = np.asarray(in1, np.float32)
        m = np.maximum(a, b)
        k = np.arange(a.shape[-1], dtype=np.float32)
        idx2v = (np.float32(imm2) + np.float32(c1) * k).astype(np.float32)
        bwin = (b > a).astype(np.float32)
        body = ((m - np.float32(c0)) + (idx2v + bwin)).astype(np.float32)
        acc = body.max(axis=-1, keepdims=True)
        return body, acc

    spec = Spec(body=body, accum=AluOp.MAX, reference=ref)
    shas = {
        ver: DveOpSpec(
            name="PACKMAX_FOLD2_ANT", opcode=1, uops=lower(spec, ver=ver),
            rd1_en=True,
        ).sha(ver)
        for ver in ("v3", "v4")
    }
    op = dve_ops.DveOp("PACKMAX_FOLD2_ANT", spec, subdim=False, uops_sha=shas)
    if all(o.name != op.name for o in dve_ops.OPS):
        dve_ops.OPS.append(op)
        dve_ops.CUSTOM_DVE_SPECS[op.name] = op.spec
        dve_ops._SUB_OPCODE_FOR_NAME[op.name] = (
            dve_ops._CUSTOM_DVE_ROW_BASE + len(dve_ops.OPS) - 1
        )
    _OP_CACHE["op"] = op
    return op


def _build():
    from contextlib import ExitStack

    import concourse.mybir as mybir
    from concourse import bacc
    from concourse.bass import IndirectOffsetOnAxis
    from concourse.tile import TileContext

    f32 = mybir.dt.float32
    bf16 = mybir.dt.bfloat16
    u32 = mybir.dt.uint32
    AF = mybir.ActivationFunctionType
    packmax_op = _get_packmax_op()

    nc = bacc.Bacc(None, target_bir_lowering=False)
    xT = nc.dram_tensor("xT", [KAUG, N], bf16, kind="ExternalInput")
    yT = nc.dram_tensor("yT", [KAUG, N], bf16, kind="ExternalInput")
    # gather tables, permuted so the packed index decodes directly to a row
    cat_p2 = nc.dram_tensor("cat_p2", [N, 2 * D], f32, kind="ExternalInput")
    cat_g2 = nc.dram_tensor("cat_g2", [N, 2 * D], f32, kind="ExternalInput")
    # host-pretransposed [p][t][c] natural-order copies
    natp_h = nc.dram_tensor("natp_h", [128, NT * 2 * D], f32, kind="ExternalInput")
    natg_h = nc.dram_tensor("natg_h", [128, NT * 2 * D], f32, kind="ExternalInput")
    loss = nc.dram_tensor("loss", [1, 1], f32, kind="ExternalOutput")

    with TileContext(nc) as tc:
        with ExitStack() as ctx:
            const = ctx.enter_context(tc.tile_pool(name="const", bufs=1))
            staged_pool = ctx.enter_context(tc.tile_pool(name="staged", bufs=6))
            scr_pool = ctx.enter_context(tc.tile_pool(name="scr", bufs=2))
            kl_pool = ctx.enter_context(tc.tile_pool(name="klp", bufs=6))
            small = ctx.enter_context(tc.tile_pool(name="small", bufs=4))
            # 2-bank psum chunks, double buffered: 4+4 banks
            fold_ps = ctx.enter_context(tc.tile_pool(name="fps", bufs=2, space="PSUM"))
            stage_ps = ctx.enter_context(tc.tile_pool(name="sps", bufs=2, space="PSUM"))

            # stationary operands duplicated at base_partition 64: row-group
            # tiling runs two K=35 matmuls concurrently in PE quadrants
            xT_sb = const.tile([64 + KAUG, N], bf16, tag="xT_sb")
            yT_sb = const.tile([64 + KAUG, N], bf16, tag="yT_sb")
            nat_p = const.tile([128, NT, 2 * D], f32, tag="nat_p")
            nat_g = const.tile([128, NT, 2 * D], f32, tag="nat_g")
            gath_g = const.tile([128, NT, 2 * D], f32, tag="gath_g")
            gath_p = const.tile([128, NT, 2 * D], f32, tag="gath_p")
            pargs_a = const.tile([128, 2, NT], f32, tag="pargs_a")
            pargs_b = const.tile([128, 2, NT], f32, tag="pargs_b")
            args_a = const.tile([128, NT], u32, tag="args_a")
            args_b = const.tile([128, NT], u32, tag="args_b")
            klacc = const.tile([128, NT], f32, tag="klacc")

            # --- input loads: column-chunked across 4 DGE queues so the
            # first tile's matmuls start ~2.5us in (vs ~30us serial) ---
            def ld(q, sb, dram, lo, hi, part):
                p0 = 0 if part == 0 else 64
                q.dma_start(out=sb[p0 : p0 + KAUG, lo:hi], in_=dram[:, lo:hi])

            # tile-0-critical 512-col pieces first (one per queue slot);
            # the moving halves alternate lo/hi by 512-block parity
            ld(nc.sync,   xT_sb, xT, 0, 512, 0)
            ld(nc.scalar, xT_sb, xT, 0, 512, 1)
            ld(nc.gpsimd, yT_sb, yT, 2560, 3072, 1)
            ld(nc.sync,   yT_sb, yT, 2048, 2560, 0)
            ld(nc.scalar, yT_sb, yT, 3072, 3584, 0)
            ld(nc.gpsimd, yT_sb, yT, 3584, 4096, 1)
            ld(nc.sync,   yT_sb, yT, 0, 512, 0)
            ld(nc.scalar, yT_sb, yT, 512, 1024, 1)
            ld(nc.gpsimd, yT_sb, yT, 1024, 1536, 0)
            ld(nc.sync,   yT_sb, yT, 1536, 2048, 1)
            # remaining halves (stationary use for layout B + other parity)
            ld(nc.scalar, yT_sb, yT, 2048, 2560, 1)
            ld(nc.gpsimd, yT_sb, yT, 2560, 3072, 0)
            ld(nc.sync,   yT_sb, yT, 3072, 3584, 1)
            ld(nc.scalar, yT_sb, yT, 3584, 4096, 0)
            ld(nc.gpsimd, yT_sb, yT, 512, 1024, 0)
            ld(nc.sync,   yT_sb, yT, 0, 512, 1)
            ld(nc.scalar, yT_sb, yT, 1024, 1536, 1)
            ld(nc.gpsimd, yT_sb, yT, 1536, 2048, 0)
            ld(nc.sync,   xT_sb, xT, 512, 2048, 0)
            ld(nc.scalar, xT_sb, xT, 512, 2048, 1)
            ld(nc.gpsimd, xT_sb, xT, 2048, 4096, 0)
            ld(nc.sync,   xT_sb, xT, 2048, 4096, 1)
            # nat tables last: not needed until the first KL slot (~40us in);
            # they would otherwise steal DMA bandwidth from the loads that
            # gate the pipeline
            nc.scalar.dma_start(out=nat_p[:, 0:16, :], in_=natp_h[:, 0 : 16 * 2 * D])
            nc.gpsimd.dma_start(
                out=nat_p[:, 16:NT, :], in_=natp_h[:, 16 * 2 * D : NT * 2 * D]
            )
            nc.sync.dma_start(out=nat_g[:, 0:16, :], in_=natg_h[:, 0 : 16 * 2 * D])
            nc.scalar.dma_start(
                out=nat_g[:, 16:NT, :], in_=natg_h[:, 16 * 2 * D : NT * 2 * D]
            )

            def mm_pair(ps, stat_sb, mov_sb, tcol, mlo, first):
                """One 1024-col chunk: two concurrent 512-col matmuls in PE
                row groups {0,1} (partitions 0..34) and {2,3} (64..98)."""
                i1 = nc.tensor.matmul(
                    ps[:, 0:512],
                    lhsT=stat_sb[0:KAUG, tcol : tcol + 128],
                    rhs=mov_sb[0:KAUG, mlo : mlo + 512],
                    start=True, stop=True,
                )
                i2 = nc.tensor.matmul(
                    ps[:, 512:1024],
                    lhsT=stat_sb[64 : 64 + KAUG, tcol : tcol + 128],
                    rhs=mov_sb[64 : 64 + KAUG, mlo + 512 : mlo + 1024],
                    start=True, stop=True,
                )
                if not first:
                    i1.ins.ldweights = False
                    i2.ins.ldweights = False

            GT = 8  # tiles per decode/gather/KL group (amortizes Pool's
                    # ~300ns fixed cost per op and the ~1.1us fixed cost
                    # per indirect-DMA instruction)

            MI = 1.5 * 2.0**23       # integer-grid rint magic
            CH = 0.5 - 2.0**-13      # floor shift, tie-safe for |q| < 2048

            def decode(pargs, args, lo, hi, on_vec=False):
                """pargs[:, 0:2, lo:hi] (packed) -> args[:, lo:hi] (u32).
                idx = pack - 4096*floor(pack/4096); floor via the fp32
                magic-number rint of (pack/4096 - CH).  Runs entirely on
                Pool (TT max via relu; the cast via a convert-copy) so no
                in-order Vector-queue slot ever waits on Pool."""
                n = hi - lo
                e = nc.vector if on_vec else nc.gpsimd
                t1 = small.tile([128, n], f32, tag="dt1")
                t2 = small.tile([128, n], f32, tag="dt2")
                if on_vec:
                    e.tensor_max(t1[:, :], pargs[:, 0, lo:hi], pargs[:, 1, lo:hi])
                else:
                    # max(a,b) = a + relu(b - a); Pool has no TT-max opcode
                    e.tensor_sub(t1[:, :], pargs[:, 1, lo:hi], pargs[:, 0, lo:hi])
                    e.tensor_relu(t1[:, :], t1[:, :])
                    e.tensor_add(t1[:, :], t1[:, :], pargs[:, 0, lo:hi])
                e.tensor_scalar(
                    t2[:, :], t1[:, :], 1.0 / 4096.0, CH,
                    op0=mybir.AluOpType.mult, op1=mybir.AluOpType.subtract,
                )
                e.tensor_scalar(
                    t2[:, :], t2[:, :], MI, MI,
                    op0=mybir.AluOpType.add, op1=mybir.AluOpType.subtract,
                )
                e.tensor_scalar_mul(t2[:, :], t2[:, :], -4096.0)
                e.tensor_add(t2[:, :], t1[:, :], t2[:, :])
                e.tensor_copy(args[:, lo:hi], t2[:, :])

            def gathers(args, table, gath, lo, hi):
                # per-tile gathers: multi-index offset APs gather garbage on
                # real DGE hardware (verified on-device)
                for j in range(lo, hi):
                    nc.gpsimd.indirect_dma_start(
                        gath[:, j, :],
                        None,
                        table[:, :],
                        IndirectOffsetOnAxis(ap=args[:, j : j + 1], axis=0),
                    )

            # KL chain for one group, staged so every op's producers ran
            # periods earlier (no head-of-line blocking on in-order queues):
            #   S = sum_d (t1 - exp(t1) - (mu_p-mu_o)^2 * exp(-lv_o))
            kl_state = {}

            def kl1(pv, ov, key, lo, hi, vec):
                e = nc.vector if vec else nc.gpsimd
                n = hi - lo
                k1 = kl_pool.tile([128, n, D], f32, tag="k1")
                kl_state[key] = (k1,)
                e.tensor_sub(k1[:, :, :], pv[:, lo:hi, D : 2 * D], ov[:, lo:hi, D : 2 * D])

            def kl2(pv, ov, key, lo, hi, vec):
                (k1,) = kl_state[key]
                n = hi - lo
                k2 = kl_pool.tile([128, n, D], f32, tag="k2")
                k3 = kl_pool.tile([128, n, D], f32, tag="k3")
                nc.scalar.activation(k2[:, :, :], k1[:, :, :], AF.Exp)
                nc.scalar.activation(k3[:, :, :], ov[:, lo:hi, D : 2 * D], AF.Exp, scale=-1.0)
                kl_state[key] = (k1, k2, k3)

            def kl3(pv, ov, key, lo, hi, vec):
                k1, k2, k3 = kl_state[key]
                e = nc.vector if vec else nc.gpsimd
                n = hi - lo
                k4 = kl_pool.tile([128, n, D], f32, tag="k4")
                e.tensor_sub(k1[:, :, :], k1[:, :, :], k2[:, :, :])
                e.tensor_sub(k2[:, :, :], pv[:, lo:hi, 0:D], ov[:, lo:hi, 0:D])
                e.tensor_mul(k2[:, :, :], k2[:, :, :], k2[:, :, :])
                e.tensor_mul(k2[:, :, :], k2[:, :, :], k3[:, :, :])
                e.tensor_sub(k4[:, :, :], k1[:, :, :], k2[:, :, :])
                kl_state[key] = (k4,)

            def kl4(key, lo, hi, first):
                (k4,) = kl_state.pop(key)
                n = hi - lo
                if first:
                    nc.vector.reduce_sum(
                        klacc[:, lo:hi], k4[:, :, :], axis=mybir.AxisListType.X
                    )
                else:
                    red = small.tile([128, n], f32, tag="red")
                    nc.vector.reduce_sum(
                        red[:, :], k4[:, :, :], axis=mybir.AxisListType.X
                    )
                    nc.gpsimd.tensor_add(klacc[:, lo:hi], klacc[:, lo:hi], red[:, :])

            # --- two layout passes, one global slot-scheduled pipeline ---
            LAYOUTS = (
                (xT_sb, yT_sb, pargs_a, args_a, cat_g2, gath_g, nat_p, gath_g, True),
                (yT_sb, xT_sb, pargs_b, args_b, cat_p2, gath_p, gath_p, nat_g, False),
            )
            sched = {}

            def at(T, fn):
                sched.setdefault(T, []).append(fn)

            NTOT = 2 * NT
            for L, (stat_sb, mov_sb, pargs, args, table, gath, pv, ov, first_kl) in (
                enumerate(LAYOUTS)
            ):
                for t in range(NT):
                    T = L * NT + t
                    tcol = t * 128
                    # stage-half chunks (cols 2048:4096) -> psum -> ACT -> SBUF
                    st = []
                    for c in range(2):
                        sps = stage_ps.tile([128, NC], f32, tag="sps")
                        mm_pair(sps, stat_sb, mov_sb, tcol, 2048 + NC * c, first=(c == 0))
                        st.append(sps)
                    sg = []
                    for c in range(2):
                        sb = staged_pool.tile([128, NC], f32, tag="staged")
                        nc.scalar.copy(out=sb[:, :], in_=st[c][:, :])
                        sg.append(sb)
                    # fold-half chunks (cols 0:2048) stay in PSUM
                    for c in range(2):
                        fps = fold_ps.tile([128, NC], f32, tag="fps")
                        mm_pair(fps, stat_sb, mov_sb, tcol, NC * c, first=False)
                        scr = scr_pool.tile([128, NC], f32, tag="scr")
                        nc.vector._custom_dve(
                            packmax_op,
                            out=scr[:, :],
                            in0=fps[:, :],
                            in1=sg[c][:, :],
                            s0=MAGIC3,
                            s1=2.0,
                            imm2=float(2048 * c),
                            accum_out=pargs[:, c, t : t + 1],
                        )
                    # run work scheduled for this slot (producers long done)
                    for fn in sched.pop(T, []):
                        fn()
                    in_last_group = L == 1 and t >= NT - GT
                    if t % GT == GT - 1 and not in_last_group:
                        g = t // GT
                        lo, hi = GT * g, GT * g + GT
                        decode(pargs, args, lo, hi)
                        gathers(args, table, gath, lo, hi)
                        key = (L, g)
                        P, O, F = pv, ov, first_kl
                        # slot spacing: gathers+decode occupy GPS for
                        # ~4.3 periods after the boundary, so kl1 (GPS,
                        # queue-ordered behind them) and kl2 (ACT) land at
                        # T+6; kl3 pops just before the next boundary's
                        # gathers enter the GPS queue; kl4 after kl3 ran.
                        at(T + 6, lambda P=P, O=O, k=key, a=lo, b=hi: kl1(P, O, k, a, b, False))
                        at(T + 6, lambda P=P, O=O, k=key, a=lo, b=hi: kl2(P, O, k, a, b, False))
                        at(T + 8, lambda P=P, O=O, k=key, a=lo, b=hi: kl3(P, O, k, a, b, False))
                        at(T + 10, lambda k=key, a=lo, b=hi, F=F: kl4(k, a, b, F))
                    elif in_last_group and t % 4 == 3:
                        # final group: split gathers in half and run the KL
                        # on the (soon idle) Vector engine to cut the tail
                        half = (t % GT) // 4
                        lo = NT - GT + 4 * half
                        hi = lo + 4
                        decode(pargs, args, lo, hi, on_vec=(half == 1))
                        gathers(args, table, gath, lo, hi)

            # drain: remaining scheduled KL work, then the last group's KL
            # entirely on Vector/ACT (Pool is still busy with its gathers)
            for T in range(NTOT, NTOT + 17):
                for fn in sched.pop(T, []):
                    fn()
            stat_sb, mov_sb, pargs, args, table, gath, pv, ov, first_kl = LAYOUTS[1]
            lo, hi = GT * 3, GT * 4
            key = (1, 3)
            kl1(pv, ov, key, lo, hi, True)
            kl2(pv, ov, key, lo, hi, True)
            kl3(pv, ov, key, lo, hi, True)
            kl4(key, lo, hi, first_kl)

            # --- final reduction ---
            # fold the two "+ sum_d 1 = +D" constants (one per side)
            nc.vector.tensor_scalar_add(klacc[:, :], klacc[:, :], float(2 * D))
            ones_col = const.tile([128, 1], f32, tag="ones_col")
            nc.vector.memset(ones_col[:, :], 1.0)
            ps_fin = stage_ps.tile([128, NC], f32, tag="sps")
            nc.tensor.matmul(
                ps_fin[0:1, 0:NT],
                lhsT=ones_col[:, :],
                rhs=klacc[:, :],
                start=True, stop=True,
            )
            fin = small.tile([1, 1], f32, tag="fin")
            nc.vector.reduce_sum(
                fin[:, :], ps_fin[0:1, 0:NT], axis=mybir.AxisListType.X
            )
            # loss = 0.5*(l1+l2), each l = -0.5*S  ->  -0.25*(S1+S2)
            nc.vector.tensor_scalar_mul(fin[:, :], fin[:, :], -0.25)
            nc.sync.dma_start(out=loss[:, :], in_=fin[:, :])

    nc.finalize()
    return nc


def _get_nc():
    if "nc" not in _NC_CACHE:
        _NC_CACHE["nc"] = _build()
    return _NC_CACHE["nc"]


def _host_prep(mu_p, lv_p, mu_g, lv_g):
    """Per-sample input marshalling.

    xT carries the 2^17 scale (exact exponent shift in bf16) and yT the
    magic partner row, so psum = MAGIC3 + 2^17*M rounded to a 4096 grid.
    Gather tables are permuted so packed idx2 = 2048c + 2q + b maps to
    row idx2 directly (orig col = 1024c + q + 2048b)."""
    import ml_dtypes

    bf16 = ml_dtypes.bfloat16
    x = mu_p.astype(bf16)
    y = mu_g.astype(bf16)
    xf = x.astype(np.float32)
    yf = y.astype(np.float32)
    ax = (-0.5 * np.sum(xf * xf, -1)).astype(bf16).astype(np.float32)
    ay = (-0.5 * np.sum(yf * yf, -1)).astype(bf16).astype(np.float32)
    ones = np.ones((N,), np.float32)
    xT = np.ascontiguousarray(
        (np.concatenate(
            [xf.T, ax[None, :], ones[None, :], ones[None, :]], 0
        ) * np.float32(ALPHA)).astype(bf16)
    )
    yT = np.ascontiguousarray(
        np.concatenate(
            [yf.T, ones[None, :], ay[None, :], np.full((1, N), GAMMA, np.float32)], 0
        ).astype(bf16)
    )
    P = np.arange(N)
    c = P >> 11
    r = P & 2047
    orig = 1024 * c + (r >> 1) + 2048 * (r & 1)
    cat_p = np.concatenate([mu_p, lv_p], 1).astype(np.float32)
    cat_g = np.concatenate([mu_g, lv_g], 1).astype(np.float32)
    natp_h = np.ascontiguousarray(
        cat_p.reshape(NT, 128, 2 * D).transpose(1, 0, 2).reshape(128, -1)
    )
    natg_h = np.ascontiguousarray(
        cat_g.reshape(NT, 128, 2 * D).transpose(1, 0, 2).reshape(128, -1)
    )
    return {
        "xT": xT,
        "yT": yT,
        "cat_p2": np.ascontiguousarray(cat_p[orig]),
        "cat_g2": np.ascontiguousarray(cat_g[orig]),
        "natp_h": natp_h,
        "natg_h": natg_h,
    }


def make_in_maps(mu_preds, logvar_preds, mu_gts, logvar_gts):
    mu_preds = np.asarray(mu_preds, dtype=np.float32)
    logvar_preds = np.asarray(logvar_preds, dtype=np.float32)
    mu_gts = np.asarray(mu_gts, dtype=np.float32)
    logvar_gts = np.asarray(logvar_gts, dtype=np.float32)
    return [
        _host_prep(mu_preds[b], logvar_preds[b], mu_gts[b], logvar_gts[b])
        for b in range(BS)
    ]


def run(in_maps, trace=False):
    from concourse.bass_utils import run_bass_kernel_spmd

    nc = _get_nc()
    res = run_bass_kernel_spmd(nc, in_maps, list(range(BS)), trace=trace)
    out = np.array(
        [np.asarray(res.results[b]["loss"]).reshape(()) for b in range(BS)],
        dtype=np.float32,
    )
    return out, res


def kernel(mu_preds, logvar_preds, mu_gts, logvar_gts):
    in_maps = make_in_maps(mu_preds, logvar_preds, mu_gts, logvar_gts)
    out, _ = run(in_maps)
    return out
